# revision 40
# baseline (speedup 1.0000x reference)
"""BitNet-style quantized linear on 8 Trainium2 NeuronCores.

Reference semantics (all f32):
    act_scale = 127 / clip(max|x| per row, 1e-5)          # [T,1]
    qx  = clip(round(x * act_scale), -128, 127)           # int8 values
    w_scale = 1 / clip(mean|weight|, 1e-5)                # scalar
    qw  = clip(round(weight * w_scale), -1, 1)            # ternary
    acc = qx @ qw.T                                       # exact int accum
    out = acc / act_scale / w_scale + bias

Sharding: data-parallel over the token dim — core c gets x[c*2048:(c+1)*2048],
weight/bias replicated.  The weight is passed pre-transposed ([in,out] layout,
a pure host-side layout change) so the contraction dim lands on SBUF
partitions for both matmul operands.

Device pipeline per core (T=2048 tokens, K=N=1024), built from few large ops
to keep per-op and semaphore overheads down:
  - weight prep: 4 chunk DMAs (1 MiB each) on the scalar ring.  Per chunk,
    one DVE scalar_tensor_tensor emits |w| (max(w*-1, w)) with column-sum
    accumulation while ACT emits sign(w).  After a partition all-reduce
    produces mean|w|, one DVE op per chunk finishes the ternary weight:
        qw = (|w| >= 0.5*mean|w|) * sign(w)
    (identical to clip(round(w/mean|w|), -1, 1) away from exact-tie .5s).
  - x pipeline in 512-token blocks (one 2 MiB DMA each on the sync ring):
    one DVE abs-max reduce [128,4,1024] -> [128,4] per block + batched scale
    ops; per 128-token subtile ACT quantizes (x*s + MAGIC, then -MAGIC to
    bf16); per 256-token supertile ONE dma-xbar transpose [128,2048] ->
    [128,16,128] on the scalar ring.
  - per subtile: 16 bf16 matmuls (FD=512) accumulate into a 2-bank PSUM
    tile [128,1024]; one fused DVE op does out = (psum * rs) + bias.
  - stores: one 1 MiB DMA per supertile on the sync ring (queued after all
    x loads, so neither ring ever head-of-line blocks the other phase).

bf16 is exact here: |qx| <= 127 and qw in {-1,0,1} are exactly representable,
products are ints <= 127, and the f32 PSUM accumulation of <= 1024 of them
stays far below 2^24.
"""

from contextlib import ExitStack

import numpy as np

import concourse.bass as bass
import concourse.mybir as mybir
import concourse.tile as tile
from concourse import bacc, bass_isa
from concourse.bass_utils import run_bass_kernel_spmd

N_CORES = 8
T_FULL, K, N = 16384, 1024, 1024
T_SHARD = T_FULL // N_CORES          # 2048 tokens per core
N_SUPER = T_SHARD // 256             # 8 super-tiles of 256 tokens (2 sub-tiles)
N_BLK = T_SHARD // 512               # 4 x-load blocks of 512 tokens
KC = K // 128                        # 8 contraction chunks of 128
WC = 4                               # weight DMA chunks (2 K-chunks each)
EPS = 1e-5
MAGIC = 12582912.0                   # 1.5 * 2^23: adding then subtracting
                                     # rounds f32 to nearest int (RNE)
F32 = mybir.dt.float32
BF16 = mybir.dt.bfloat16


def build_kernel(nc, tc, ctx):
    x = nc.dram_tensor("x", [T_SHARD, K], F32, kind="ExternalInput").ap()
    wt = nc.dram_tensor("wt", [K, N], F32, kind="ExternalInput").ap()
    bias = nc.dram_tensor("bias", [N], F32, kind="ExternalInput").ap()
    out = nc.dram_tensor("out", [T_SHARD, N], F32, kind="ExternalOutput").ap()

    consts = ctx.enter_context(tc.tile_pool(name="consts", bufs=1))
    wload = ctx.enter_context(tc.tile_pool(name="wload", bufs=1))
    wpool = ctx.enter_context(tc.tile_pool(name="wpool", bufs=1))
    xpool = ctx.enter_context(tc.tile_pool(name="xpool", bufs=5))
    qpool = ctx.enter_context(tc.tile_pool(name="qpool", bufs=2))
    # qxt lives until mm_deq(st) three supertiles later; 4 bufs so the
    # transpose for st+3 never WAR-waits on st's matmuls
    qxpool = ctx.enter_context(tc.tile_pool(name="qxpool", bufs=4))
    opool = ctx.enter_context(tc.tile_pool(name="opool", bufs=2))
    small = ctx.enter_context(tc.tile_pool(name="small", bufs=8))
    psum = ctx.enter_context(tc.tile_pool(name="psum", bufs=4, space="PSUM"))

    # ---- constants ----------------------------------------------------
    # bias broadcast to all 128 partitions (stride-0 partition dim DMA)
    bias_bc = consts.tile([128, N], F32)
    bias_bcast_ap = bass.AP(
        tensor=bias.tensor, offset=bias.offset, ap=[[0, 128]] + list(bias.ap)
    )
    nc.gpsimd.dma_start(out=bias_bc, in_=bias_bcast_ap)

    # PE warm-up: ~96 throwaway matmuls keep the HAM activity monitor at
    # K=8/8 (2.4 GHz) through the ~30us head, so the real matmul stream
    # starts warm instead of at half clock.  They finish right as the
    # first quantized weights appear.
    warm = consts.tile([128, 512], BF16)
    nc.vector.memset(warm, 0.0)
    wpm = psum.tile([128, N], F32, tag="pm")
    for _ in range(96):
        nc.tensor.matmul(wpm[:, :512], warm[:, :128], warm)

    # constant MAGIC tile: lets the -MAGIC pass run as a GpSimd
    # tensor_tensor (full-tensor operands are fast on GpSimd; vector-scalar
    # operands are not), taking one of the two quant passes off ACT
    magic_bc = consts.tile([128, K], F32)
    nc.vector.memset(magic_bc, MAGIC)

    # ---- weight prep --------------------------------------------------
    wabs = wpool.tile([128, KC, N], F32, tag="wabs")
    sgn = wpool.tile([128, KC, N], BF16, tag="sgn")
    qwt = wpool.tile([128, KC, N], BF16, tag="qwt")
    wsums = consts.tile([128, WC], F32)
    wcs = []

    def w_load(c):
        wc = wload.tile([128, 2, N], F32, tag=f"wc{c}")
        rows = wt[c * 256:(c + 1) * 256, :].rearrange("(g p) n -> p g n", p=128)
        nc.sync.dma_start(out=wc, in_=rows)
        wcs.append(wc)

    def w_stats(c):
        # |w| = max(w*-1, w) with column-sum accum on DVE while ACT does
        # sign(w): the two 2.3us ops chase each chunk arrival in parallel
        nc.vector.scalar_tensor_tensor(
            out=wabs[:, 2 * c:2 * c + 2, :], in0=wcs[c], scalar=-1.0,
            in1=wcs[c],
            op0=mybir.AluOpType.mult, op1=mybir.AluOpType.max,
            accum_out=wsums[:, c:c + 1],
        )
        nc.scalar.activation(
            out=sgn[:, 2 * c:2 * c + 2, :], in_=wcs[c],
            func=mybir.ActivationFunctionType.Sign,
        )

    def w_scale():
        # total |w| sum -> all partitions; tau = 0.5*clip(mean,eps)
        wsum_tot = consts.tile([128, 1], F32)
        nc.vector.reduce_sum(wsum_tot, wsums, axis=mybir.AxisListType.X)
        allsum = consts.tile([128, 1], F32)
        nc.gpsimd.partition_all_reduce(
            allsum, wsum_tot, channels=128, reduce_op=bass_isa.ReduceOp.add
        )
        mwc = consts.tile([128, 1], F32)      # clip(mean|w|, eps)
        nc.vector.tensor_scalar(
            mwc, allsum, float(2.0 ** -20), EPS,
            op0=mybir.AluOpType.mult, op1=mybir.AluOpType.max,
        )
        tau = consts.tile([128, 1], F32)      # ternary threshold
        nc.vector.tensor_scalar_mul(tau, mwc, 0.5)
        mw127 = consts.tile([128, 1], F32)    # clip(mean)/127 (dequant factor)
        nc.vector.tensor_scalar_mul(mw127, mwc, 1.0 / 127.0)
        return tau, mw127

    def w_quant(c, tau):
        # ternary quantize: qw = (|w| >= tau) * sign(w) on DVE, in fine
        # [128,1024] pieces so the PE can start consuming chunk 0 early
        nc.vector.scalar_tensor_tensor(
            out=qwt[:, c, :], in0=wabs[:, c, :],
            scalar=tau, in1=sgn[:, c, :],
            op0=mybir.AluOpType.is_ge, op1=mybir.AluOpType.mult,
        )

    # ---- x pipeline ---------------------------------------------------
    def load_x(st):
        rows = x[st * 256:(st + 1) * 256, :].rearrange("(a p) k -> p a k", p=128)
        xt = xpool.tile([128, 2, K], F32, tag="xt")
        nc.sync.dma_start(out=xt, in_=rows)
        return xt

    def preproc(st, xt):
        """scales + quantize + transpose one supertile; returns (qxt, mc2)"""
        mraw = small.tile([128, 2], F32, tag="mraw")
        nc.vector.reduce_max(
            mraw, xt, axis=mybir.AxisListType.X, apply_absolute_value=True
        )
        mc2 = small.tile([128, 2], F32, tag="mc2")
        nc.vector.tensor_scalar_max(mc2, mraw, EPS)
        rcp2 = small.tile([128, 2], F32, tag="rcp2")
        nc.vector.reciprocal(rcp2, mc2)
        s2 = small.tile([128, 2], F32, tag="s2")     # act_scale = 127/mc
        nc.vector.tensor_scalar_mul(s2, rcp2, 127.0)

        t1 = qpool.tile([128, 2, K], F32, tag="t1")
        qx = qpool.tile([128, 2, K], BF16, tag="qx")
        for a in range(2):
            nc.scalar.activation(
                out=t1[:, a, :], in_=xt[:, a, :],
                func=mybir.ActivationFunctionType.Copy,
                bias=MAGIC, scale=s2[:, a:a + 1],
            )
            nc.gpsimd.tensor_tensor(
                qx[:, a, :], t1[:, a, :], magic_bc,
                op=mybir.AluOpType.subtract,
            )
        # one xbar transpose for the whole supertile: free index f=a*1024+k
        # lands at chunk f//128 = a*8 + c, i.e. qxt[:, a*8+c, :] is
        # (subtile a, k-chunk c) with k on partitions.
        qxt = qxpool.tile([128, 2 * KC, 128], BF16, tag="qxt")
        nc.sync.dma_start_transpose(qxt, qx)
        return qxt, mc2

    def mm_deq(st, pre, mw127):
        """matmuls + fused dequant/bias into ostage; returns ostage"""
        qxt, mc2 = pre
        rs2 = small.tile([128, 2], F32, tag="rs2")   # dequant row scales
        nc.vector.tensor_scalar_mul(rs2, mc2, mw127)
        ostage = opool.tile([128, 2, N], F32, tag="ostage")
        for a in range(2):
            pm = psum.tile([128, N], F32, tag="pm")   # 2 PSUM banks
            # c outer / h inner: consecutive matmul pairs share the same
            # stationary operand, halving the LDWEIGHTS traffic
            for c in range(KC):
                for h in range(2):
                    nc.tensor.matmul(
                        pm[:, h * 512:(h + 1) * 512],
                        qxt[:, a * KC + c, :],
                        qwt[:, c, h * 512:(h + 1) * 512],
                        start=(c == 0),
                        stop=(c == KC - 1),
                    )
            # out = (psum * rs) + bias in one DVE op over both banks
            nc.vector.scalar_tensor_tensor(
                out=ostage[:, a, :], in0=pm, scalar=rs2[:, a:a + 1],
                in1=bias_bc,
                op0=mybir.AluOpType.mult, op1=mybir.AluOpType.add,
            )
        return ostage

    def store(st, ostage):
        orows = out[st * 256:(st + 1) * 256, :].rearrange(
            "(a p) n -> p a n", p=128
        )
        nc.gpsimd.dma_start(out=orows, in_=ostage)

    # ---- emission schedule --------------------------------------------
    # sync ring: ALL loads, strictly ordered x0, w0..w3, x1, x2, ... —
    # same-ring DMAs run FIFO at full rate, so the head-critical weight
    # finishes early instead of round-robin-sharing HBM with x tiles.
    # scalar ring: per-supertile transposes + stores (both dep-gated
    # trailing ops, so neither blocks a load).
    # Head ring order (sync, FIFO): x0, w0-w2, tr0, w3, x1, tr1, x2 — the
    # supertile-0 transpose slots in before the last weight chunk so the
    # first matmul group is never transpose-gated, and x1 lands right when
    # its preproc becomes the critical chain.
    xts, pres = {}, {}
    xts[0] = load_x(0)
    for c in range(WC - 1):
        w_load(c)
    pres[0] = preproc(0, xts.pop(0))
    w_load(WC - 1)
    xts[1] = load_x(1)

    for c in range(WC):
        w_stats(c)
    tau, mw127 = w_scale()
    # preproc(1) is emitted BETWEEN the tau chain and the qw burst: its
    # reduce+smalls take priority over the 11us of qw quantization on DVE,
    # so supertile 1's matmuls can follow supertile 0 without a gap.
    pres[1] = preproc(1, xts.pop(1))
    for c in range(KC):
        w_quant(c, tau)

    LA_LOAD, LA_PRE = 4, 3
    for st in range(2, LA_LOAD):
        xts[st] = load_x(st)
    for st in range(LA_PRE - 1, LA_PRE):
        pres[st] = preproc(st, xts.pop(st))
    for st in range(N_SUPER):
        if st + LA_LOAD < N_SUPER:
            xts[st + LA_LOAD] = load_x(st + LA_LOAD)
        ostage = mm_deq(st, pres.pop(st), mw127)
        if st + LA_PRE < N_SUPER:
            pres[st + LA_PRE] = preproc(st + LA_PRE, xts.pop(st + LA_PRE))
        store(st, ostage)


_CACHE = {}


def _get_compiled():
    if "nc" not in _CACHE:
        nc = bacc.Bacc(
            "TRN2", target_bir_lowering=False, debug=False, num_devices=N_CORES
        )
        with tile.TileContext(nc) as tc:
            with ExitStack() as ctx:
                build_kernel(nc, tc, ctx)
        nc.compile()
        _CACHE["nc"] = nc
    return _CACHE["nc"]


def kernel_with_results(x, weight, bias, trace=False):
    assert x.shape == (T_FULL, K) and weight.shape == (N, K)
    x = np.ascontiguousarray(np.asarray(x, dtype=np.float32))
    wt = np.ascontiguousarray(np.asarray(weight, dtype=np.float32).T)
    bias = np.ascontiguousarray(np.asarray(bias, dtype=np.float32))

    nc = _get_compiled()
    in_maps = [
        {"x": x[c * T_SHARD:(c + 1) * T_SHARD], "wt": wt, "bias": bias}
        for c in range(N_CORES)
    ]
    res = run_bass_kernel_spmd(nc, in_maps, list(range(N_CORES)), trace=trace)
    out = np.concatenate([res.results[c]["out"] for c in range(N_CORES)], axis=0)
    return out, res


def kernel(x, weight, bias):
    out, _ = kernel_with_results(x, weight, bias)
    return out
